# revision 54
# baseline (speedup 1.0000x reference)
"""Trainium2 Bass kernel for multi-head attention (B=2, S=2048, D=1024, H=16).

Sharding: 8 cores = 2 (batch, data-parallel) x 4 (head-groups, tensor-parallel).
Each core (b, g) handles batch b and heads [4g, 4g+4) (a 256-wide slice of the
model dim), computing a partial output contribution; the host sums the 4
head-group partials per batch and adds the output bias.

Per-core pipeline (everything bf16 into fp32 PSUM accumulation):
  - projections: qp^T/kp^T feature-major ([d, s], W^T stationary) so the
    attention matmuls need no transposes; vp sequence-major with a fused
    bias row and a ones column per head (the ones column makes attn@V
    emit the softmax row-sums for free as PSUM row 64).
  - attention, per (head-pair, q-block of 512): scores^T via two
    row-packed K=64 matmuls per k-tile (two heads run concurrently on
    the 128-row PE array); exp(x/8) on the scalar engine straight out of
    a 2-bank PSUM group; attn@V accumulates [65, 512] per head in PSUM.
  - normalization: one fast DVE copy releases the accumulator, then
    reciprocal_approx_fast + gpsimd partition-broadcast + multiply.
  - output projection is cut into 8 units per q-block and interleaved
    into the next q-block's groups so its matmuls, PSUM->SBUF copies and
    output DMA all hide under the exp pipeline.
  - the projections themselves are mostly emitted *inside* the first
    attention pair (k-tile kt only needs kp j-block kt//4), hiding the
    projection phase under the exp pipeline's startup.
The scalar engine (softmax exp: 16.8M elements/core at 1 elem/lane/cycle
plus 352-cycle instruction overhead) and the PE array (8.6 GFLOP/core
plus un-hidable LDWEIGHTS, ldw-opt is off in this toolchain) are both
near-saturated at ~146us and ~195us busy respectively; measured
end-to-end ~228us per core.
"""

import os
import numpy as np
import ml_dtypes

import concourse.bass as bass
import concourse.bacc as bacc
import concourse.mybir as mybir
import concourse.tile as tile
from concourse.bass_utils import run_bass_kernel_spmd

F32 = mybir.dt.float32
F32R = mybir.dt.float32r
BF16 = mybir.dt.bfloat16
AF = mybir.ActivationFunctionType

B, S, D = 2, 2048, 1024
H, DK = 16, 64
G = 4                  # head-groups (tensor parallel across cores)
DG = D // G            # 256 features per core
HPG = H // G           # 4 heads per core (2 row-packed pairs)
VEXT = HPG * (DK + 1)  # 260: per head [64 vp dims | 1 ones column]
P = 128
N_CORES = 8

_NC = None


def _build_program():
    nc = bacc.Bacc("TRN2", target_bir_lowering=False)
    qT = nc.dram_tensor("qT", [D, S], BF16, kind="ExternalInput")
    kT = nc.dram_tensor("kT", [D, S], BF16, kind="ExternalInput")
    vT = nc.dram_tensor("vT", [D, S], BF16, kind="ExternalInput")
    wqT = nc.dram_tensor("wqT", [D, DG], BF16, kind="ExternalInput")
    wkT = nc.dram_tensor("wkT", [D, DG], BF16, kind="ExternalInput")
    wvm = nc.dram_tensor("wvm", [D, VEXT], BF16, kind="ExternalInput")
    wvb = nc.dram_tensor("wvb", [1, VEXT], BF16, kind="ExternalInput")
    onesr = nc.dram_tensor("onesr", [1, P], BF16, kind="ExternalInput")
    woT = nc.dram_tensor("woT", [DG, D], BF16, kind="ExternalInput")
    bqv = nc.dram_tensor("bqv", [DG], F32, kind="ExternalInput")
    bkv = nc.dram_tensor("bkv", [DG], F32, kind="ExternalInput")
    out = nc.dram_tensor("out", [S, D], F32, kind="ExternalOutput")

    with tile.TileContext(nc) as tc:
        _body(nc, tc, qT, kT, vT, wqT, wkT, wvm, wvb, onesr, woT, bqv, bkv, out)
    nc.compile()
    return nc


def _body(nc, tc, qT, kT, vT, wqT, wkT, wvm, wvb, onesr, woT, bqv, bkv, out):
    with (
        tc.tile_pool(name="consts", bufs=1) as consts,
        tc.tile_pool(name="persist", bufs=1) as persist,
        tc.tile_pool(name="stage", bufs=6) as stage,
        tc.tile_pool(name="etp", bufs=8) as etp,
        tc.tile_pool(name="small", bufs=4) as small,
        tc.tile_pool(name="outp", bufs=8) as outp,
        tc.tile_pool(name="psA", bufs=2, space="PSUM") as psA,
        tc.tile_pool(name="psG", bufs=2, space="PSUM") as psG,
        tc.tile_pool(name="psC", bufs=1, space="PSUM") as psC,
    ):
        # --- constants / weights ---
        wk_sb = consts.tile([P, 8, DG], BF16)
        nc.scalar.dma_start(wk_sb[:], wkT[:].rearrange("(t p) m -> p t m", p=P))
        wv_sb = consts.tile([P, 8, VEXT], BF16)
        nc.scalar.dma_start(wv_sb[:], wvm[:].rearrange("(t p) m -> p t m", p=P))
        wvb_sb = consts.tile([1, VEXT], BF16)
        nc.scalar.dma_start(wvb_sb[:], wvb[:])
        bk_sb = consts.tile([P, 2], F32)
        nc.scalar.dma_start(bk_sb[:], bkv[:].rearrange("(t p) -> p t", p=P))
        ones_sb = consts.tile([1, P], BF16)
        nc.scalar.dma_start(ones_sb[:], onesr[:])

        # bias+ones row broadcast once to all partitions (folded into the
        # vp PSUM->SBUF copy as a vector add, replacing 16 K=1 matmuls)
        wvb_bc = consts.tile([P, VEXT], BF16)
        nc.gpsimd.partition_broadcast(wvb_bc[:], wvb_sb[:])

        # warm the ACT exp table early so the ~2.7us load overlaps phase 1
        warm = consts.tile([1, 8], F32)
        nc.vector.memset(warm[:], 0.0)
        nc.scalar.activation(warm[:], warm[:], AF.Exp)

        # --- persistent activations ---
        qpT_sb = persist.tile([P, 2, S], BF16)   # [d%128, d-tile(=pair), s]
        kpT_sb = persist.tile([P, 2, S], BF16)
        vp_sb = persist.tile([P, 16, VEXT], BF16)  # [s%128, s-tile, 4*(64+1)]
        an_sb = persist.tile([P, 2, S], BF16)   # normalized attn output^T

        GRP = 2  # PSUM banks per exp group (one kt, both heads)
        # --- phase 1a+1b interleaved: kp^T first (phase 2 needs all of it),
        # vp interleaved for DMA/PE overlap, then qp^T j-blocks which are
        # emitted inside the attention loop (q-block qb only needs slice j=qb).
        def ps_alloc(n, i=[0]):
            i[0] += 1
            if i[0] % 2:
                return psA.tile([P, 512], F32, tag="a", name="ps_mm")[:, :n]
            return psG.tile([P, GRP * 512], F32, tag="g", name="gps")[:, :n]

        proj_xb = {}

        def proj_dma(src_t, j):
            xb = stage.tile([P, 8, 512], BF16, tag="xb", name="xb")
            nc.sync.dma_start(
                xb[:],
                src_t[:].rearrange("(t p) s -> p t s", p=P)[
                    :, :, j * 512 : (j + 1) * 512
                ],
            )
            return xb

        def proj_half(src_t, w_sb, b_sb, dst, j, dt):
            key = (id(src_t), j)
            if key not in proj_xb:
                proj_xb[key] = proj_dma(src_t, j)
            xb = proj_xb[key]
            ps = ps_alloc(512)
            for kt in range(8):
                nc.tensor.matmul(
                    ps[:],
                    lhsT=w_sb[:, kt, dt * P : (dt + 1) * P],
                    rhs=xb[:, kt, :],
                    start=(kt == 0),
                    stop=(kt == 7),
                )
            nc.vector.tensor_scalar_add(
                dst[:, dt, j * 512 : (j + 1) * 512], ps[:], b_sb[:, dt : dt + 1]
            )

        def proj_block(src_t, w_sb, b_sb, dst, j):
            for dt in range(2):
                proj_half(src_t, w_sb, b_sb, dst, j, dt)

        vtb_cache = {}

        def vp_block(st):
            # two s-tiles per DMA: 1KB bursts instead of 512B, half the loads
            st0 = st - st % 2
            if st0 not in vtb_cache:
                vtb2 = stage.tile([P, 8, 2 * P], BF16, tag="vtb", name="vtb")
                nc.sync.dma_start(
                    vtb2[:],
                    vT[:].rearrange("(t p) s -> p t s", p=P)[
                        :, :, st0 * P : (st0 + 2) * P
                    ],
                )
                vtb_cache[st0] = vtb2
            vtb = vtb_cache[st0]
            off = (st - st0) * P
            psv = ps_alloc(VEXT)
            for kt in range(8):
                nc.tensor.matmul(
                    psv[:],
                    lhsT=vtb[:, kt, off : off + P],
                    rhs=wv_sb[:, kt, :],
                    start=(kt == 0),
                    stop=(kt == 7),
                )
            nc.vector.tensor_tensor(
                vp_sb[:, st, :], psv[:], wvb_bc[:], mybir.AluOpType.add
            )

        wq_sb = consts.tile([P, 8, DG], BF16)
        nc.scalar.dma_start(wq_sb[:], wqT[:].rearrange("(t p) m -> p t m", p=P))
        bq_sb = consts.tile([P, 2], F32)
        nc.scalar.dma_start(bq_sb[:], bqv[:].rearrange("(t p) -> p t", p=P))
        wo_sb = consts.tile([P, 2, D], BF16)
        nc.scalar.dma_start(wo_sb[:], woT[:].rearrange("(t p) o -> p t o", p=P))

        # bootstrap: just enough of kp/qp/vp for (qb0, pair0, kt=0,1);
        # the rest of the projections are emitted inside qb0/pair0 below,
        # hiding their PE time under the exp pipeline instead of idling ACT
        proj_half(kT, wk_sb, bk_sb, kpT_sb, 0, 0)
        proj_half(qT, wq_sb, bq_sb, qpT_sb, 0, 0)
        vp_block(0)
        vp_block(1)

        # insertion schedule for qb0/pair0: at group kt, emit these blocks
        fuse0 = {kt: [] for kt in range(16)}
        for kt in range(14):
            fuse0[kt].append(("vp", kt + 2))
        for j in (1, 2, 3):
            fuse0[4 * j - 2].append(("kp", j, 0))   # needed at group 4j
        for j in (0, 1, 2, 3):
            fuse0[[2, 6, 10, 13][j]].append(("kp", j, 1))  # for pair1
        fuse0[12].append(("qp", 0, 1))              # qp j0 dt1 for pair1

        # --- phase 2 per q-block; qp^T j-block emitted just-in-time ---
        def d_unit(qb, u, split_ring=False):
            # one (q-tile, out-half) unit of the output projection for block qb
            qt, o = u // 2, u % 2
            q0 = qb * 512 + qt * P
            dps = psA.tile([P, 512], F32, tag="a", name="dps")
            for p2 in range(2):
                nc.tensor.matmul(
                    dps[:],
                    lhsT=an_sb[:, p2, q0 : q0 + P],
                    rhs=wo_sb[:, p2, o * 512 : (o + 1) * 512],
                    start=(p2 == 0),
                    stop=(p2 == 1),
                )
            osb = outp.tile([P, 512], F32, tag="o")
            nc.vector.tensor_copy(osb[:], dps[:])
            eng = nc.scalar if (split_ring and o) else nc.sync
            eng.dma_start(out[q0 : q0 + P, o * 512 : (o + 1) * 512], osb[:])

        def d_block(qb):
            for u in range(8):
                d_unit(qb, u, split_ring=True)

        for qb in range(4):
            qs = slice(qb * 512, (qb + 1) * 512)
            for pair in range(2):
                cc = psC.tile([DK + 1, 1024], F32, tag="c", name="cc")
                c_ps = [cc[:, :512], cc[:, 512:]]
                # spread the previous q-block's output projection through this
                # pair's groups so the scalar engine never starves
                d_units = list(range(4)) if qb > 0 else []

                def c_mms(kt, et):
                    for hh in range(2):
                        h = 2 * pair + hh
                        nc.tensor.matmul(
                            c_ps[hh],
                            lhsT=vp_sb[:, kt, h * (DK + 1) : (h + 1) * (DK + 1)],
                            rhs=et[:, hh * 512 : (hh + 1) * 512],
                            start=(kt == 0),
                            stop=(kt == 15),
                        )

                for kt in range(16):
                    gps = psG.tile([P, GRP * 512], F32, tag="g", name="gps")
                    for hh in range(2):
                        hp = slice(hh * DK, (hh + 1) * DK)
                        nc.tensor.matmul(
                            gps[:, hh * 512 : (hh + 1) * 512],
                            lhsT=kpT_sb[hp, pair, kt * P : (kt + 1) * P],
                            rhs=qpT_sb[hp, pair, qs],
                            start=True,
                            stop=True,
                        )
                    et = etp.tile([P, GRP * 512], BF16, tag="e", name="et")
                    nc.scalar.activation(
                        et[:], gps[:], AF.Exp, scale=1.0 / np.sqrt(DK)
                    )
                    c_mms(kt, et)
                    if d_units and kt in (3, 7, 11, 14):
                        d_unit(qb - 1, 4 * pair + d_units.pop(0))
                    if qb == 0 and pair == 0:
                        for item in fuse0[kt]:
                            if item[0] == "vp":
                                vp_block(item[1])
                            elif item[0] == "kp":
                                proj_half(kT, wk_sb, bk_sb, kpT_sb, item[1], item[2])
                            else:
                                proj_half(qT, wq_sb, bq_sb, qpT_sb, item[1], item[2])
                    if pair == 1 and qb < 3 and kt in (1, 9):
                        proj_half(qT, wq_sb, bq_sb, qpT_sb, qb + 1, kt // 8)
                # single fast copy releases the PSUM accumulator; normalize
                # (reciprocal of row 64, broadcast, multiply) runs from SBUF.
                # The very last pair has no successor waiting on the banks, so
                # skip the staging copy and read PSUM directly (shorter chain
                # in front of the final output-projection block).
                last = qb == 3 and pair == 1
                if last:
                    csrc, coff = cc, [slice(0, 512), slice(512, 1024)]
                else:
                    csb = small.tile([DK + 1, 1024], F32, tag="csb")
                    nc.vector.tensor_copy(csb[:], cc[:])
                    csrc, coff = csb, [slice(0, 512), slice(512, 1024)]
                for hh in range(2):
                    cs = coff[hh]
                    rsum = small.tile([1, 512], F32, tag="rsum")
                    nc.vector.tensor_copy(rsum[:], csrc[DK : DK + 1, cs])
                    rinv = small.tile([1, 512], F32, tag="rinv")
                    nc.vector.reciprocal_approx_fast(rinv[:], rsum[:])
                    rbc = small.tile([DK, 512], F32, tag="rbc")
                    nc.gpsimd.partition_broadcast(rbc[:], rinv[:])
                    nc.vector.tensor_tensor(
                        an_sb[hh * DK : (hh + 1) * DK, pair, qs],
                        csrc[:DK, cs],
                        rbc[:],
                        mybir.AluOpType.mult,
                    )




        d_block(3)


def _get_program():
    global _NC
    if _NC is None:
        _NC = _build_program()
    return _NC


def _make_in_maps(v, k, q, Wv, bv, Wk, bk, Wq, bq, Wo, bo):
    f32 = np.float32
    bf16 = ml_dtypes.bfloat16
    qT = [np.ascontiguousarray(q[b].T).astype(bf16) for b in range(B)]
    kT = [np.ascontiguousarray(k[b].T).astype(bf16) for b in range(B)]
    vT = [np.ascontiguousarray(v[b].T).astype(bf16) for b in range(B)]

    per_group = []
    for g in range(G):
        gs = slice(g * DG, (g + 1) * DG)
        wqT = np.ascontiguousarray(Wq[gs, :].T).astype(bf16)
        wkT = np.ascontiguousarray(Wk[gs, :].T).astype(bf16)
        wvm = np.zeros((D, VEXT), dtype=f32)
        wvb = np.zeros((1, VEXT), dtype=f32)
        for h in range(HPG):
            cs = slice(h * (DK + 1), h * (DK + 1) + DK)
            rows = slice(g * DG + h * DK, g * DG + (h + 1) * DK)
            wvm[:, cs] = Wv[rows, :].T
            wvb[0, cs] = bv[rows]
            wvb[0, h * (DK + 1) + DK] = 1.0
        wvm = wvm.astype(bf16)
        wvb = wvb.astype(bf16)
        woT = np.ascontiguousarray(Wo[:, gs].T).astype(bf16)
        per_group.append(
            dict(
                wqT=wqT,
                wkT=wkT,
                wvm=wvm,
                wvb=wvb,
                woT=woT,
                bqv=np.ascontiguousarray(bq[gs], dtype=f32),
                bkv=np.ascontiguousarray(bk[gs], dtype=f32),
            )
        )

    in_maps = []
    for c in range(N_CORES):
        b, g = c // G, c % G
        m = dict(qT=qT[b], kT=kT[b], vT=vT[b],
                 onesr=np.ones((1, P), dtype=bf16), **per_group[g])
        in_maps.append(m)
    return in_maps


def _gather(results, bo):
    out = np.zeros((B, S, D), dtype=np.float32)
    for c in range(N_CORES):
        b = c // G
        out[b] += results[c]["out"]
    out += bo.astype(np.float32)
    return out


def run(v, k, q, Wv, bv, Wk, bk, Wq, bq, Wo, bo, trace=False):
    nc = _get_program()
    in_maps = _make_in_maps(v, k, q, Wv, bv, Wk, bk, Wq, bq, Wo, bo)
    res = run_bass_kernel_spmd(
        nc, in_maps, core_ids=list(range(N_CORES)), trace=trace
    )
    return _gather(res.results, np.asarray(bo)), res


def kernel(v, k, q, Wv, bv, Wk, bk, Wq, bq, Wo, bo):
    args = [np.asarray(x, dtype=np.float32)
            for x in (v, k, q, Wv, bv, Wk, bk, Wq, bq, Wo, bo)]
    out, _ = run(*args, trace=bool(int(os.environ.get("MHA_TRACE", "0"))))
    return out


# revision 55
# speedup vs baseline: 1.0416x; 1.0416x over previous
"""Trainium2 Bass kernel for multi-head attention (B=2, S=2048, D=1024, H=16).

Sharding: 8 cores = 2 (batch, data-parallel) x 4 (head-groups, tensor-parallel).
Each core (b, g) handles batch b and heads [4g, 4g+4) (a 256-wide slice of the
model dim), computing a partial output contribution; the host sums the 4
head-group partials per batch and adds the output bias.

Per-core pipeline (everything bf16 into fp32 PSUM accumulation):
  - projections: qp^T/kp^T feature-major ([d, s], W^T stationary) so the
    attention matmuls need no transposes; vp sequence-major with a fused
    bias row and a ones column per head (the ones column makes attn@V
    emit the softmax row-sums for free as PSUM row 64).
  - attention, per (head-pair, q-block of 512): scores^T via two
    row-packed K=64 matmuls per k-tile (two heads run concurrently on
    the 128-row PE array); exp(x/8) on the scalar engine straight out of
    a 2-bank PSUM group; attn@V accumulates [65, 512] per head in PSUM.
  - normalization: one fast DVE copy releases the accumulator, then
    reciprocal_approx_fast + gpsimd partition-broadcast + multiply.
  - output projection is cut into 8 units per q-block and interleaved
    into the next q-block's groups so its matmuls, PSUM->SBUF copies and
    output DMA all hide under the exp pipeline.
  - the projections themselves are mostly emitted *inside* the first
    attention pair (k-tile kt only needs kp j-block kt//4), hiding the
    projection phase under the exp pipeline's startup.
The scalar engine (softmax exp: 16.8M elements/core at 1 elem/lane/cycle
plus 352-cycle instruction overhead) and the PE array (8.6 GFLOP/core
plus un-hidable LDWEIGHTS, ldw-opt is off in this toolchain) are both
near-saturated at ~146us and ~195us busy respectively; measured
end-to-end ~228us per core.
"""

import os
import numpy as np
import ml_dtypes

import concourse.bass as bass
import concourse.bacc as bacc
import concourse.mybir as mybir
import concourse.tile as tile
from concourse.bass_utils import run_bass_kernel_spmd

F32 = mybir.dt.float32
F32R = mybir.dt.float32r
BF16 = mybir.dt.bfloat16
AF = mybir.ActivationFunctionType

B, S, D = 2, 2048, 1024
H, DK = 16, 64
G = 4                  # head-groups (tensor parallel across cores)
DG = D // G            # 256 features per core
HPG = H // G           # 4 heads per core (2 row-packed pairs)
VEXT = HPG * (DK + 1)  # 260: per head [64 vp dims | 1 ones column]
P = 128
N_CORES = 8

_NC = None


def _build_program():
    nc = bacc.Bacc("TRN2", target_bir_lowering=False)
    qT = nc.dram_tensor("qT", [D, S], BF16, kind="ExternalInput")
    kT = nc.dram_tensor("kT", [D, S], BF16, kind="ExternalInput")
    vT = nc.dram_tensor("vT", [D, S], BF16, kind="ExternalInput")
    wqT = nc.dram_tensor("wqT", [D, DG], BF16, kind="ExternalInput")
    wkT = nc.dram_tensor("wkT", [D, DG], BF16, kind="ExternalInput")
    wvm = nc.dram_tensor("wvm", [D, VEXT], BF16, kind="ExternalInput")
    wvb = nc.dram_tensor("wvb", [1, VEXT], BF16, kind="ExternalInput")
    onesr = nc.dram_tensor("onesr", [1, P], BF16, kind="ExternalInput")
    woT = nc.dram_tensor("woT", [DG, D], BF16, kind="ExternalInput")
    bqv = nc.dram_tensor("bqv", [DG], F32, kind="ExternalInput")
    bkv = nc.dram_tensor("bkv", [DG], F32, kind="ExternalInput")
    out = nc.dram_tensor("out", [S, D], F32, kind="ExternalOutput")

    with tile.TileContext(nc) as tc:
        _body(nc, tc, qT, kT, vT, wqT, wkT, wvm, wvb, onesr, woT, bqv, bkv, out)
    nc.compile()
    return nc


def _body(nc, tc, qT, kT, vT, wqT, wkT, wvm, wvb, onesr, woT, bqv, bkv, out):
    with (
        tc.tile_pool(name="consts", bufs=1) as consts,
        tc.tile_pool(name="persist", bufs=1) as persist,
        tc.tile_pool(name="stage", bufs=6) as stage,
        tc.tile_pool(name="etp", bufs=8) as etp,
        tc.tile_pool(name="small", bufs=4) as small,
        tc.tile_pool(name="outp", bufs=8) as outp,
        tc.tile_pool(name="psA", bufs=2, space="PSUM") as psA,
        tc.tile_pool(name="psG", bufs=2, space="PSUM") as psG,
        tc.tile_pool(name="psC", bufs=1, space="PSUM") as psC,
    ):
        # --- constants / weights ---
        wk_sb = consts.tile([P, 8, DG], BF16)
        nc.scalar.dma_start(wk_sb[:], wkT[:].rearrange("(t p) m -> p t m", p=P))
        wv_sb = consts.tile([P, 8, VEXT], BF16)
        nc.scalar.dma_start(wv_sb[:], wvm[:].rearrange("(t p) m -> p t m", p=P))
        wvb_sb = consts.tile([1, VEXT], BF16)
        nc.scalar.dma_start(wvb_sb[:], wvb[:])
        bk_sb = consts.tile([P, 2], F32)
        nc.scalar.dma_start(bk_sb[:], bkv[:].rearrange("(t p) -> p t", p=P))
        ones_sb = consts.tile([1, P], BF16)
        nc.scalar.dma_start(ones_sb[:], onesr[:])

        # bias+ones row broadcast once to all partitions (folded into the
        # vp PSUM->SBUF copy as a vector add, replacing 16 K=1 matmuls)
        wvb_bc = consts.tile([P, VEXT], BF16)
        nc.gpsimd.partition_broadcast(wvb_bc[:], wvb_sb[:])

        # warm the ACT exp table early so the ~2.7us load overlaps phase 1
        warm = consts.tile([1, 8], F32)
        nc.vector.memset(warm[:], 0.0)
        nc.scalar.activation(warm[:], warm[:], AF.Exp)

        # --- persistent activations ---
        qpT_sb = persist.tile([P, 2, S], BF16)   # [d%128, d-tile(=pair), s]
        kpT_sb = persist.tile([P, 2, S], BF16)
        vp_sb = persist.tile([P, 16, VEXT], BF16)  # [s%128, s-tile, 4*(64+1)]
        an_sb = persist.tile([P, 2, S], BF16)   # normalized attn output^T

        GRP = 2  # PSUM banks per exp group (one kt, both heads)
        # --- phase 1a+1b interleaved: kp^T first (phase 2 needs all of it),
        # vp interleaved for DMA/PE overlap, then qp^T j-blocks which are
        # emitted inside the attention loop (q-block qb only needs slice j=qb).
        def ps_alloc(n, i=[0]):
            i[0] += 1
            if i[0] % 2:
                return psA.tile([P, 512], F32, tag="a", name="ps_mm")[:, :n]
            return psG.tile([P, GRP * 512], F32, tag="g", name="gps")[:, :n]

        proj_xb = {}

        def proj_dma(src_t, j):
            xb = stage.tile([P, 8, 512], BF16, tag="xb", name="xb")
            nc.sync.dma_start(
                xb[:],
                src_t[:].rearrange("(t p) s -> p t s", p=P)[
                    :, :, j * 512 : (j + 1) * 512
                ],
            )
            return xb

        def proj_half(src_t, w_sb, b_sb, dst, j, dt):
            key = (id(src_t), j)
            if key not in proj_xb:
                proj_xb[key] = proj_dma(src_t, j)
            xb = proj_xb[key]
            ps = ps_alloc(512)
            for kt in range(8):
                nc.tensor.matmul(
                    ps[:],
                    lhsT=w_sb[:, kt, dt * P : (dt + 1) * P],
                    rhs=xb[:, kt, :],
                    start=(kt == 0),
                    stop=(kt == 7),
                )
            nc.vector.tensor_scalar_add(
                dst[:, dt, j * 512 : (j + 1) * 512], ps[:], b_sb[:, dt : dt + 1]
            )

        def proj_block(src_t, w_sb, b_sb, dst, j):
            for dt in range(2):
                proj_half(src_t, w_sb, b_sb, dst, j, dt)

        vtb_cache = {}

        def vp_block(st):
            # two s-tiles per DMA: 1KB bursts instead of 512B, half the loads
            st0 = st - st % 2
            if st0 not in vtb_cache:
                vtb2 = stage.tile([P, 8, 2 * P], BF16, tag="vtb", name="vtb")
                nc.sync.dma_start(
                    vtb2[:],
                    vT[:].rearrange("(t p) s -> p t s", p=P)[
                        :, :, st0 * P : (st0 + 2) * P
                    ],
                )
                vtb_cache[st0] = vtb2
            vtb = vtb_cache[st0]
            off = (st - st0) * P
            psv = ps_alloc(VEXT)
            for kt in range(8):
                nc.tensor.matmul(
                    psv[:],
                    lhsT=vtb[:, kt, off : off + P],
                    rhs=wv_sb[:, kt, :],
                    start=(kt == 0),
                    stop=(kt == 7),
                )
            nc.vector.tensor_tensor(
                vp_sb[:, st, :], psv[:], wvb_bc[:], mybir.AluOpType.add
            )

        wq_sb = consts.tile([P, 8, DG], BF16)
        nc.scalar.dma_start(wq_sb[:], wqT[:].rearrange("(t p) m -> p t m", p=P))
        bq_sb = consts.tile([P, 2], F32)
        nc.scalar.dma_start(bq_sb[:], bqv[:].rearrange("(t p) -> p t", p=P))
        wo_sb = consts.tile([P, 2, D], BF16)
        nc.scalar.dma_start(wo_sb[:], woT[:].rearrange("(t p) o -> p t o", p=P))

        # bootstrap: just enough of kp/qp/vp for (qb0, pair0, kt=0,1);
        # the rest of the projections are emitted inside qb0/pair0 below,
        # hiding their PE time under the exp pipeline instead of idling ACT
        proj_half(kT, wk_sb, bk_sb, kpT_sb, 0, 0)
        proj_half(qT, wq_sb, bq_sb, qpT_sb, 0, 0)
        vp_block(0)
        vp_block(1)

        # insertion schedule for qb0/pair0: at group kt, emit these blocks
        fuse0 = {kt: [] for kt in range(16)}
        for kt in range(14):
            fuse0[kt].append(("vp", kt + 2))
        for j in (1, 2, 3):
            fuse0[4 * j - 2].append(("kp", j, 0))   # needed at group 4j
        for j in (0, 1, 2, 3):
            fuse0[[2, 6, 10, 13][j]].append(("kp", j, 1))  # for pair1
        fuse0[12].append(("qp", 0, 1))              # qp j0 dt1 for pair1

        # --- phase 2 per q-block; qp^T j-block emitted just-in-time ---
        def d_unit(qb, u, split_ring=False):
            # one (q-tile, out-half) unit of the output projection for block qb
            qt, o = u // 2, u % 2
            q0 = qb * 512 + qt * P
            dps = psA.tile([P, 512], F32, tag="a", name="dps")
            for p2 in range(2):
                nc.tensor.matmul(
                    dps[:],
                    lhsT=an_sb[:, p2, q0 : q0 + P],
                    rhs=wo_sb[:, p2, o * 512 : (o + 1) * 512],
                    start=(p2 == 0),
                    stop=(p2 == 1),
                )
            osb = outp.tile([P, 512], F32, tag="o")
            nc.vector.tensor_copy(osb[:], dps[:])
            eng = nc.scalar if (split_ring and o) else nc.sync
            eng.dma_start(out[q0 : q0 + P, o * 512 : (o + 1) * 512], osb[:])

        def d_block(qb):
            for u in range(8):
                d_unit(qb, u, split_ring=True)

        for qb in range(4):
            qs = slice(qb * 512, (qb + 1) * 512)
            for pair in range(2):
                cc = psC.tile([DK + 1, 1024], F32, tag="c", name="cc")
                c_ps = [cc[:, :512], cc[:, 512:]]
                # spread the previous q-block's output projection through this
                # pair's groups so the scalar engine never starves
                d_units = list(range(4)) if qb > 0 else []

                def c_mms(kt, et):
                    for hh in range(2):
                        h = 2 * pair + hh
                        nc.tensor.matmul(
                            c_ps[hh],
                            lhsT=vp_sb[:, kt, h * (DK + 1) : (h + 1) * (DK + 1)],
                            rhs=et[:, hh * 512 : (hh + 1) * 512],
                            start=(kt == 0),
                            stop=(kt == 15),
                        )

                for kt in range(16):
                    # prefetch the next q-block's qp input early so its
                    # projection at pair1 never waits behind this block's
                    # output DMAs on the sync ring (traced 2-4us stalls)
                    if pair == 0 and qb < 3 and kt == (8 if qb == 0 else 0):
                        pkey = (id(qT), qb + 1)
                        if pkey not in proj_xb:
                            proj_xb[pkey] = proj_dma(qT, qb + 1)
                    gps = psG.tile([P, GRP * 512], F32, tag="g", name="gps")
                    for hh in range(2):
                        hp = slice(hh * DK, (hh + 1) * DK)
                        nc.tensor.matmul(
                            gps[:, hh * 512 : (hh + 1) * 512],
                            lhsT=kpT_sb[hp, pair, kt * P : (kt + 1) * P],
                            rhs=qpT_sb[hp, pair, qs],
                            start=True,
                            stop=True,
                        )
                    et = etp.tile([P, GRP * 512], BF16, tag="e", name="et")
                    nc.scalar.activation(
                        et[:], gps[:], AF.Exp, scale=1.0 / np.sqrt(DK)
                    )
                    c_mms(kt, et)
                    if d_units and kt in (3, 7, 11, 14):
                        d_unit(qb - 1, 4 * pair + d_units.pop(0))
                    if qb == 0 and pair == 0:
                        for item in fuse0[kt]:
                            if item[0] == "vp":
                                vp_block(item[1])
                            elif item[0] == "kp":
                                proj_half(kT, wk_sb, bk_sb, kpT_sb, item[1], item[2])
                            else:
                                proj_half(qT, wq_sb, bq_sb, qpT_sb, item[1], item[2])
                    if pair == 1 and qb < 3 and kt in (1, 9):
                        proj_half(qT, wq_sb, bq_sb, qpT_sb, qb + 1, kt // 8)
                # single fast copy releases the PSUM accumulator; normalize
                # (reciprocal of row 64, broadcast, multiply) runs from SBUF.
                # The very last pair has no successor waiting on the banks, so
                # skip the staging copy and read PSUM directly (shorter chain
                # in front of the final output-projection block).
                last = qb == 3 and pair == 1
                if last:
                    csrc, coff = cc, [slice(0, 512), slice(512, 1024)]
                else:
                    csb = small.tile([DK + 1, 1024], F32, tag="csb")
                    nc.vector.tensor_copy(csb[:], cc[:])
                    csrc, coff = csb, [slice(0, 512), slice(512, 1024)]
                for hh in range(2):
                    cs = coff[hh]
                    rsum = small.tile([1, 512], F32, tag="rsum")
                    nc.vector.tensor_copy(rsum[:], csrc[DK : DK + 1, cs])
                    rinv = small.tile([1, 512], F32, tag="rinv")
                    nc.vector.reciprocal_approx_fast(rinv[:], rsum[:])
                    rbc = small.tile([DK, 512], F32, tag="rbc")
                    nc.gpsimd.partition_broadcast(rbc[:], rinv[:])
                    nc.vector.tensor_tensor(
                        an_sb[hh * DK : (hh + 1) * DK, pair, qs],
                        csrc[:DK, cs],
                        rbc[:],
                        mybir.AluOpType.mult,
                    )




        d_block(3)


def _get_program():
    global _NC
    if _NC is None:
        _NC = _build_program()
    return _NC


def _make_in_maps(v, k, q, Wv, bv, Wk, bk, Wq, bq, Wo, bo):
    f32 = np.float32
    bf16 = ml_dtypes.bfloat16
    qT = [np.ascontiguousarray(q[b].T).astype(bf16) for b in range(B)]
    kT = [np.ascontiguousarray(k[b].T).astype(bf16) for b in range(B)]
    vT = [np.ascontiguousarray(v[b].T).astype(bf16) for b in range(B)]

    per_group = []
    for g in range(G):
        gs = slice(g * DG, (g + 1) * DG)
        wqT = np.ascontiguousarray(Wq[gs, :].T).astype(bf16)
        wkT = np.ascontiguousarray(Wk[gs, :].T).astype(bf16)
        wvm = np.zeros((D, VEXT), dtype=f32)
        wvb = np.zeros((1, VEXT), dtype=f32)
        for h in range(HPG):
            cs = slice(h * (DK + 1), h * (DK + 1) + DK)
            rows = slice(g * DG + h * DK, g * DG + (h + 1) * DK)
            wvm[:, cs] = Wv[rows, :].T
            wvb[0, cs] = bv[rows]
            wvb[0, h * (DK + 1) + DK] = 1.0
        wvm = wvm.astype(bf16)
        wvb = wvb.astype(bf16)
        woT = np.ascontiguousarray(Wo[:, gs].T).astype(bf16)
        per_group.append(
            dict(
                wqT=wqT,
                wkT=wkT,
                wvm=wvm,
                wvb=wvb,
                woT=woT,
                bqv=np.ascontiguousarray(bq[gs], dtype=f32),
                bkv=np.ascontiguousarray(bk[gs], dtype=f32),
            )
        )

    in_maps = []
    for c in range(N_CORES):
        b, g = c // G, c % G
        m = dict(qT=qT[b], kT=kT[b], vT=vT[b],
                 onesr=np.ones((1, P), dtype=bf16), **per_group[g])
        in_maps.append(m)
    return in_maps


def _gather(results, bo):
    out = np.zeros((B, S, D), dtype=np.float32)
    for c in range(N_CORES):
        b = c // G
        out[b] += results[c]["out"]
    out += bo.astype(np.float32)
    return out


def run(v, k, q, Wv, bv, Wk, bk, Wq, bq, Wo, bo, trace=False):
    nc = _get_program()
    in_maps = _make_in_maps(v, k, q, Wv, bv, Wk, bk, Wq, bq, Wo, bo)
    res = run_bass_kernel_spmd(
        nc, in_maps, core_ids=list(range(N_CORES)), trace=trace
    )
    return _gather(res.results, np.asarray(bo)), res


def kernel(v, k, q, Wv, bv, Wk, bk, Wq, bq, Wo, bo):
    args = [np.asarray(x, dtype=np.float32)
            for x in (v, k, q, Wv, bv, Wk, bk, Wq, bq, Wo, bo)]
    out, _ = run(*args, trace=bool(int(os.environ.get("MHA_TRACE", "0"))))
    return out


# revision 58
# speedup vs baseline: 1.0436x; 1.0020x over previous
"""Trainium2 Bass kernel for multi-head attention (B=2, S=2048, D=1024, H=16).

Sharding: 8 cores = 2 (batch, data-parallel) x 4 (head-groups, tensor-parallel).
Each core (b, g) handles batch b and heads [4g, 4g+4) (a 256-wide slice of the
model dim), computing a partial output contribution; the host sums the 4
head-group partials per batch and adds the output bias.

Per-core pipeline (everything bf16 into fp32 PSUM accumulation):
  - projections: qp^T/kp^T feature-major ([d, s], W^T stationary) so the
    attention matmuls need no transposes; vp sequence-major with a fused
    bias row and a ones column per head (the ones column makes attn@V
    emit the softmax row-sums for free as PSUM row 64).
  - attention, per (head-pair, q-block of 512): scores^T via two
    row-packed K=64 matmuls per k-tile (two heads run concurrently on
    the 128-row PE array); exp(x/8) on the scalar engine straight out of
    a 2-bank PSUM group; attn@V accumulates [65, 512] per head in PSUM.
  - normalization: one fast DVE copy releases the accumulator, then
    reciprocal_approx_fast + gpsimd partition-broadcast + multiply.
  - output projection is cut into 8 units per q-block and interleaved
    into the next q-block's groups so its matmuls, PSUM->SBUF copies and
    output DMA all hide under the exp pipeline.
  - the projections themselves are mostly emitted *inside* the first
    attention pair (k-tile kt only needs kp j-block kt//4), hiding the
    projection phase under the exp pipeline's startup.
The scalar engine (softmax exp: 16.8M elements/core at 1 elem/lane/cycle
plus 352-cycle instruction overhead) and the PE array (8.6 GFLOP/core
plus un-hidable LDWEIGHTS, ldw-opt is off in this toolchain) are both
near-saturated at ~146us and ~195us busy respectively; measured
end-to-end ~228us per core.
"""

import os
import numpy as np
import ml_dtypes

import concourse.bass as bass
import concourse.bacc as bacc
import concourse.mybir as mybir
import concourse.tile as tile
from concourse.bass_utils import run_bass_kernel_spmd

F32 = mybir.dt.float32
F32R = mybir.dt.float32r
BF16 = mybir.dt.bfloat16
AF = mybir.ActivationFunctionType

B, S, D = 2, 2048, 1024
H, DK = 16, 64
G = 4                  # head-groups (tensor parallel across cores)
DG = D // G            # 256 features per core
HPG = H // G           # 4 heads per core (2 row-packed pairs)
VEXT = HPG * (DK + 1)  # 260: per head [64 vp dims | 1 ones column]
P = 128
N_CORES = 8

_NC = None


def _build_program():
    nc = bacc.Bacc("TRN2", target_bir_lowering=False)
    qT = nc.dram_tensor("qT", [D, S], BF16, kind="ExternalInput")
    kT = nc.dram_tensor("kT", [D, S], BF16, kind="ExternalInput")
    vT = nc.dram_tensor("vT", [D, S], BF16, kind="ExternalInput")
    wqT = nc.dram_tensor("wqT", [D, DG], BF16, kind="ExternalInput")
    wkT = nc.dram_tensor("wkT", [D, DG], BF16, kind="ExternalInput")
    wvm = nc.dram_tensor("wvm", [D, VEXT], BF16, kind="ExternalInput")
    wvb = nc.dram_tensor("wvb", [1, VEXT], BF16, kind="ExternalInput")
    onesr = nc.dram_tensor("onesr", [1, P], BF16, kind="ExternalInput")
    woT = nc.dram_tensor("woT", [DG, D], BF16, kind="ExternalInput")
    bqv = nc.dram_tensor("bqv", [DG], F32, kind="ExternalInput")
    bkv = nc.dram_tensor("bkv", [DG], F32, kind="ExternalInput")
    out = nc.dram_tensor("out", [S, D], BF16, kind="ExternalOutput")

    with tile.TileContext(nc) as tc:
        _body(nc, tc, qT, kT, vT, wqT, wkT, wvm, wvb, onesr, woT, bqv, bkv, out)
    nc.compile()
    return nc


def _body(nc, tc, qT, kT, vT, wqT, wkT, wvm, wvb, onesr, woT, bqv, bkv, out):
    with (
        tc.tile_pool(name="consts", bufs=1) as consts,
        tc.tile_pool(name="persist", bufs=1) as persist,
        tc.tile_pool(name="stage", bufs=6) as stage,
        tc.tile_pool(name="etp", bufs=8) as etp,
        tc.tile_pool(name="small", bufs=4) as small,
        tc.tile_pool(name="outp", bufs=8) as outp,
        tc.tile_pool(name="psA", bufs=2, space="PSUM") as psA,
        tc.tile_pool(name="psG", bufs=2, space="PSUM") as psG,
        tc.tile_pool(name="psC", bufs=1, space="PSUM") as psC,
    ):
        # --- constants / weights ---
        wk_sb = consts.tile([P, 8, DG], BF16)
        nc.scalar.dma_start(wk_sb[:], wkT[:].rearrange("(t p) m -> p t m", p=P))
        wv_sb = consts.tile([P, 8, VEXT], BF16)
        nc.scalar.dma_start(wv_sb[:], wvm[:].rearrange("(t p) m -> p t m", p=P))
        wvb_sb = consts.tile([1, VEXT], BF16)
        nc.scalar.dma_start(wvb_sb[:], wvb[:])
        bk_sb = consts.tile([P, 2], F32)
        nc.scalar.dma_start(bk_sb[:], bkv[:].rearrange("(t p) -> p t", p=P))
        ones_sb = consts.tile([1, P], BF16)
        nc.scalar.dma_start(ones_sb[:], onesr[:])

        # bias+ones row broadcast once to all partitions (folded into the
        # vp PSUM->SBUF copy as a vector add, replacing 16 K=1 matmuls)
        wvb_bc = consts.tile([P, VEXT], BF16)
        nc.gpsimd.partition_broadcast(wvb_bc[:], wvb_sb[:])

        # warm the ACT exp table early so the ~2.7us load overlaps phase 1
        warm = consts.tile([1, 8], F32)
        nc.vector.memset(warm[:], 0.0)
        nc.scalar.activation(warm[:], warm[:], AF.Exp)

        # --- persistent activations ---
        qpT_sb = persist.tile([P, 2, S], BF16)   # [d%128, d-tile(=pair), s]
        kpT_sb = persist.tile([P, 2, S], BF16)
        vp_sb = persist.tile([P, 16, VEXT], BF16)  # [s%128, s-tile, 4*(64+1)]
        an_sb = persist.tile([P, 2, S], BF16)   # normalized attn output^T

        GRP = 2  # PSUM banks per exp group (one kt, both heads)
        # --- phase 1a+1b interleaved: kp^T first (phase 2 needs all of it),
        # vp interleaved for DMA/PE overlap, then qp^T j-blocks which are
        # emitted inside the attention loop (q-block qb only needs slice j=qb).
        def ps_alloc(n, i=[0]):
            i[0] += 1
            if i[0] % 2:
                return psA.tile([P, 512], F32, tag="a", name="ps_mm")[:, :n]
            return psG.tile([P, GRP * 512], F32, tag="g", name="gps")[:, :n]

        proj_xb = {}

        def proj_dma(src_t, j):
            xb = stage.tile([P, 8, 512], BF16, tag="xb", name="xb")
            nc.sync.dma_start(
                xb[:],
                src_t[:].rearrange("(t p) s -> p t s", p=P)[
                    :, :, j * 512 : (j + 1) * 512
                ],
            )
            return xb

        def proj_half(src_t, w_sb, b_sb, dst, j, dt):
            key = (id(src_t), j)
            if key not in proj_xb:
                proj_xb[key] = proj_dma(src_t, j)
            xb = proj_xb[key]
            ps = ps_alloc(512)
            for kt in range(8):
                nc.tensor.matmul(
                    ps[:],
                    lhsT=w_sb[:, kt, dt * P : (dt + 1) * P],
                    rhs=xb[:, kt, :],
                    start=(kt == 0),
                    stop=(kt == 7),
                )
            nc.vector.tensor_scalar_add(
                dst[:, dt, j * 512 : (j + 1) * 512], ps[:], b_sb[:, dt : dt + 1]
            )

        def proj_block(src_t, w_sb, b_sb, dst, j):
            for dt in range(2):
                proj_half(src_t, w_sb, b_sb, dst, j, dt)

        vtb_cache = {}

        def vp_block(st):
            # two s-tiles per DMA: 1KB bursts instead of 512B, half the loads
            st0 = st - st % 2
            if st0 not in vtb_cache:
                vtb2 = stage.tile([P, 8, 2 * P], BF16, tag="vtb", name="vtb")
                nc.sync.dma_start(
                    vtb2[:],
                    vT[:].rearrange("(t p) s -> p t s", p=P)[
                        :, :, st0 * P : (st0 + 2) * P
                    ],
                )
                vtb_cache[st0] = vtb2
            vtb = vtb_cache[st0]
            off = (st - st0) * P
            psv = ps_alloc(VEXT)
            for kt in range(8):
                nc.tensor.matmul(
                    psv[:],
                    lhsT=vtb[:, kt, off : off + P],
                    rhs=wv_sb[:, kt, :],
                    start=(kt == 0),
                    stop=(kt == 7),
                )
            nc.vector.tensor_tensor(
                vp_sb[:, st, :], psv[:], wvb_bc[:], mybir.AluOpType.add
            )

        wq_sb = consts.tile([P, 8, DG], BF16)
        nc.scalar.dma_start(wq_sb[:], wqT[:].rearrange("(t p) m -> p t m", p=P))
        bq_sb = consts.tile([P, 2], F32)
        nc.scalar.dma_start(bq_sb[:], bqv[:].rearrange("(t p) -> p t", p=P))
        wo_sb = consts.tile([P, 2, D], BF16)
        nc.scalar.dma_start(wo_sb[:], woT[:].rearrange("(t p) o -> p t o", p=P))

        # bootstrap: just enough of kp/qp/vp for (qb0, pair0, kt=0,1);
        # the rest of the projections are emitted inside qb0/pair0 below,
        # hiding their PE time under the exp pipeline instead of idling ACT
        proj_half(kT, wk_sb, bk_sb, kpT_sb, 0, 0)
        proj_half(qT, wq_sb, bq_sb, qpT_sb, 0, 0)
        vp_block(0)
        vp_block(1)

        # insertion schedule for qb0/pair0: at group kt, emit these blocks
        fuse0 = {kt: [] for kt in range(16)}
        for kt in range(14):
            fuse0[kt].append(("vp", kt + 2))
        for j in (1, 2, 3):
            fuse0[4 * j - 2].append(("kp", j, 0))   # needed at group 4j
        for j in (0, 1, 2, 3):
            fuse0[[2, 6, 10, 13][j]].append(("kp", j, 1))  # for pair1
        fuse0[12].append(("qp", 0, 1))              # qp j0 dt1 for pair1

        # --- phase 2 per q-block; qp^T j-block emitted just-in-time ---
        def d_unit(qb, u, split_ring=False):
            # one (q-tile, out-half) unit of the output projection for block qb
            qt, o = u // 2, u % 2
            q0 = qb * 512 + qt * P
            dps = psA.tile([P, 512], F32, tag="a", name="dps")
            for p2 in range(2):
                nc.tensor.matmul(
                    dps[:],
                    lhsT=an_sb[:, p2, q0 : q0 + P],
                    rhs=wo_sb[:, p2, o * 512 : (o + 1) * 512],
                    start=(p2 == 0),
                    stop=(p2 == 1),
                )
            # bf16 partials: the host sums the 4 head-group partials in
            # f32; halves the output DMA bytes (tail is output-bound)
            osb = outp.tile([P, 512], BF16, tag="o")
            nc.vector.tensor_copy(osb[:], dps[:])
            eng = nc.scalar if (split_ring and o) else nc.sync
            eng.dma_start(out[q0 : q0 + P, o * 512 : (o + 1) * 512], osb[:])

        def d_block(qb):
            for u in range(8):
                d_unit(qb, u, split_ring=True)

        for qb in range(4):
            qs = slice(qb * 512, (qb + 1) * 512)
            for pair in range(2):
                cc = psC.tile([DK + 1, 1024], F32, tag="c", name="cc")
                c_ps = [cc[:, :512], cc[:, 512:]]
                # spread the previous q-block's output projection through this
                # pair's groups so the scalar engine never starves
                d_units = list(range(4)) if qb > 0 else []

                def c_mms(kt, et):
                    for hh in range(2):
                        h = 2 * pair + hh
                        nc.tensor.matmul(
                            c_ps[hh],
                            lhsT=vp_sb[:, kt, h * (DK + 1) : (h + 1) * (DK + 1)],
                            rhs=et[:, hh * 512 : (hh + 1) * 512],
                            start=(kt == 0),
                            stop=(kt == 15),
                        )

                for kt in range(16):
                    # prefetch the next q-block's qp input early so its
                    # projection at pair1 never waits behind this block's
                    # output DMAs on the sync ring (traced 2-4us stalls)
                    if pair == 0 and qb < 3 and kt == (8 if qb == 0 else 0):
                        pkey = (id(qT), qb + 1)
                        if pkey not in proj_xb:
                            proj_xb[pkey] = proj_dma(qT, qb + 1)
                    gps = psG.tile([P, GRP * 512], F32, tag="g", name="gps")
                    for hh in range(2):
                        hp = slice(hh * DK, (hh + 1) * DK)
                        nc.tensor.matmul(
                            gps[:, hh * 512 : (hh + 1) * 512],
                            lhsT=kpT_sb[hp, pair, kt * P : (kt + 1) * P],
                            rhs=qpT_sb[hp, pair, qs],
                            start=True,
                            stop=True,
                        )
                    et = etp.tile([P, GRP * 512], BF16, tag="e", name="et")
                    nc.scalar.activation(
                        et[:], gps[:], AF.Exp, scale=1.0 / np.sqrt(DK)
                    )
                    c_mms(kt, et)
                    if d_units and kt in (3, 7, 11, 14):
                        d_unit(qb - 1, 4 * pair + d_units.pop(0))
                    if qb == 0 and pair == 0:
                        for item in fuse0[kt]:
                            if item[0] == "vp":
                                vp_block(item[1])
                            elif item[0] == "kp":
                                proj_half(kT, wk_sb, bk_sb, kpT_sb, item[1], item[2])
                            else:
                                proj_half(qT, wq_sb, bq_sb, qpT_sb, item[1], item[2])
                    if pair == 1 and qb < 3 and kt in (1, 9):
                        proj_half(qT, wq_sb, bq_sb, qpT_sb, qb + 1, kt // 8)
                # single fast copy releases the PSUM accumulator; normalize
                # (reciprocal of row 64, broadcast, multiply) runs from SBUF.
                # The very last pair has no successor waiting on the banks, so
                # skip the staging copy and read PSUM directly (shorter chain
                # in front of the final output-projection block).
                last = qb == 3 and pair == 1
                if last:
                    csrc, coff = cc, [slice(0, 512), slice(512, 1024)]
                else:
                    csb = small.tile([DK + 1, 1024], F32, tag="csb")
                    nc.vector.tensor_copy(csb[:], cc[:])
                    csrc, coff = csb, [slice(0, 512), slice(512, 1024)]
                for hh in range(2):
                    cs = coff[hh]
                    rsum = small.tile([1, 512], F32, tag="rsum")
                    nc.vector.tensor_copy(rsum[:], csrc[DK : DK + 1, cs])
                    rinv = small.tile([1, 512], F32, tag="rinv")
                    nc.vector.reciprocal_approx_fast(rinv[:], rsum[:])
                    rbc = small.tile([DK, 512], F32, tag="rbc")
                    nc.gpsimd.partition_broadcast(rbc[:], rinv[:])
                    nc.vector.tensor_tensor(
                        an_sb[hh * DK : (hh + 1) * DK, pair, qs],
                        csrc[:DK, cs],
                        rbc[:],
                        mybir.AluOpType.mult,
                    )




        d_block(3)


def _get_program():
    global _NC
    if _NC is None:
        _NC = _build_program()
    return _NC


def _make_in_maps(v, k, q, Wv, bv, Wk, bk, Wq, bq, Wo, bo):
    f32 = np.float32
    bf16 = ml_dtypes.bfloat16
    qT = [np.ascontiguousarray(q[b].T).astype(bf16) for b in range(B)]
    kT = [np.ascontiguousarray(k[b].T).astype(bf16) for b in range(B)]
    vT = [np.ascontiguousarray(v[b].T).astype(bf16) for b in range(B)]

    per_group = []
    for g in range(G):
        gs = slice(g * DG, (g + 1) * DG)
        wqT = np.ascontiguousarray(Wq[gs, :].T).astype(bf16)
        wkT = np.ascontiguousarray(Wk[gs, :].T).astype(bf16)
        wvm = np.zeros((D, VEXT), dtype=f32)
        wvb = np.zeros((1, VEXT), dtype=f32)
        for h in range(HPG):
            cs = slice(h * (DK + 1), h * (DK + 1) + DK)
            rows = slice(g * DG + h * DK, g * DG + (h + 1) * DK)
            wvm[:, cs] = Wv[rows, :].T
            wvb[0, cs] = bv[rows]
            wvb[0, h * (DK + 1) + DK] = 1.0
        wvm = wvm.astype(bf16)
        wvb = wvb.astype(bf16)
        woT = np.ascontiguousarray(Wo[:, gs].T).astype(bf16)
        per_group.append(
            dict(
                wqT=wqT,
                wkT=wkT,
                wvm=wvm,
                wvb=wvb,
                woT=woT,
                bqv=np.ascontiguousarray(bq[gs], dtype=f32),
                bkv=np.ascontiguousarray(bk[gs], dtype=f32),
            )
        )

    in_maps = []
    for c in range(N_CORES):
        b, g = c // G, c % G
        m = dict(qT=qT[b], kT=kT[b], vT=vT[b],
                 onesr=np.ones((1, P), dtype=bf16), **per_group[g])
        in_maps.append(m)
    return in_maps


def _gather(results, bo):
    out = np.zeros((B, S, D), dtype=np.float32)
    for c in range(N_CORES):
        b = c // G
        out[b] += np.asarray(results[c]["out"], dtype=np.float32)
    out += bo.astype(np.float32)
    return out


def run(v, k, q, Wv, bv, Wk, bk, Wq, bq, Wo, bo, trace=False):
    nc = _get_program()
    in_maps = _make_in_maps(v, k, q, Wv, bv, Wk, bk, Wq, bq, Wo, bo)
    res = run_bass_kernel_spmd(
        nc, in_maps, core_ids=list(range(N_CORES)), trace=trace
    )
    return _gather(res.results, np.asarray(bo)), res


def kernel(v, k, q, Wv, bv, Wk, bk, Wq, bq, Wo, bo):
    args = [np.asarray(x, dtype=np.float32)
            for x in (v, k, q, Wv, bv, Wk, bk, Wq, bq, Wo, bo)]
    out, _ = run(*args, trace=bool(int(os.environ.get("MHA_TRACE", "0"))))
    return out
